# revision 22
# baseline (speedup 1.0000x reference)
import numpy as np

# nn_GRUDirectModel on 8 trn2 cores.
# 2-layer GRU (PyTorch gate order r,z,n) + MLP head.
# B=512, T=336, E=16, H=128, FH=24, FT=4.
# Data-parallel: batch sharded 8 x 64. Per core, the recurrence runs in a
# "gate-partition" layout: hidden state h^T is [H=128 partitions, 64 batch],
# gate pre-activations live in PSUM, where chunked input-projection GEMMs
# pre-fill xp and the per-step hidden matmuls accumulate onto them.

B, T, E, H, FH, FT = 512, 336, 16, 128, 24, 4
NCORES = 8
BL = B // NCORES  # 64

_PROG_CACHE = {}


def _build_program(t_steps):
    from concourse import bass, bacc, tile

    mybir = bass.mybir
    dt = mybir.dt
    f32 = dt.float32
    bf16 = dt.bfloat16
    Alu = mybir.AluOpType
    Act = mybir.ActivationFunctionType

    npairs = t_steps // 2
    xcols = t_steps * BL

    nc = bacc.Bacc("TRN2", target_bir_lowering=False, debug=False)

    # DRAM I/O (per-core shapes). All weights packed into one tensor so a
    # matmul never waits on more than ~2 DMA queue semaphores (ISA limit).
    WPACK_COLS = 4 * 384 + 128 + 128 + 1
    d_xaug = nc.dram_tensor("xaug", [E + 1, xcols], bf16, kind="ExternalInput").ap()
    d_xfut = nc.dram_tensor("xfut", [FT, BL * FH], bf16, kind="ExternalInput").ap()
    d_wpack = nc.dram_tensor("wpack", [128, WPACK_COLS], bf16, kind="ExternalInput").ap()
    d_bias = nc.dram_tensor("bias", [H, 9], f32, kind="ExternalInput").ap()
    d_y = nc.dram_tensor("y", [1, BL * FH], f32, kind="ExternalOutput").ap()
    d_hdbg = nc.dram_tensor("hdbg", [H, BL], f32, kind="ExternalOutput").ap()
    d_hdbg0 = nc.dram_tensor("hdbg0", [H, BL], f32, kind="ExternalOutput").ap()

    with tile.TileContext(nc) as tc:
        const = tc.alloc_tile_pool(name="const", bufs=1)
        ring = tc.alloc_tile_pool(name="ring", bufs=1)
        wk = tc.alloc_tile_pool(name="wk", bufs=4)
        ps0p = tc.alloc_tile_pool(name="ps0p", bufs=2, space="PSUM")
        ps1p = tc.alloc_tile_pool(name="ps1p", bufs=2, space="PSUM")

        xaug = const.tile([E + 1, xcols], bf16)
        xfut = const.tile([FT, BL * FH], bf16)
        wpack = const.tile([128, WPACK_COLS], bf16)
        bias = const.tile([H, 9], f32)

        # gpsimd DMAs go through the single SW-DGE queue -> one semaphore, so
        # consuming matmuls never exceed the per-instruction sync-wait limit.
        nc.gpsimd.dma_start(wpack[:], d_wpack)
        nc.gpsimd.dma_start(bias[:], d_bias)
        # x in chunks so the first GEMM isn't gated on the whole transfer
        nxc = 8 if (xcols % 8 == 0 and (xcols // 8) % 128 == 0 and xcols // 8 >= 2048) else 1
        ccols = xcols // nxc
        for j in range(nxc):
            nc.gpsimd.dma_start(
                xaug[:, j * ccols : (j + 1) * ccols],
                d_xaug[:, j * ccols : (j + 1) * ccols],
            )
        nc.gpsimd.dma_start(xfut[:], d_xfut)

        whh0 = wpack[:, 0:384]
        wih1 = wpack[:, 384:768]
        whh1 = wpack[:, 768:1152]
        wih0 = wpack[0 : E + 1, 1152:1536]
        w1h = wpack[:, 1536:1664]
        w1f = wpack[0:FT, 1664:1792]
        w2 = wpack[:, 1792:1793]

        bhn0 = bias[:, 0:1]
        br1 = bias[:, 1:2]
        bz1 = bias[:, 2:3]
        bhn1 = bias[:, 3:4]
        bin1 = bias[:, 4:5]
        b1v = bias[:, 5:6]
        b2 = bias[0:1, 6:7]

        R0 = 8  # layer-0 output ring (slots of BL cols)
        R1 = 4
        h0ring = ring.tile([H, R0 * BL], bf16)
        h1ring = ring.tile([H, R1 * BL], bf16)

        pair_ps = [{}, {}]  # psum generation per (layer, pair)

        # PSUM bank col layout (fp32, 512 cols):
        #   [0:128]  r  (2 steps x 64)
        #   [128:256] z
        #   [256:384] xn   (input-proj only; never touched by hidden MM)
        #   [384:512] hn   (hidden MM only)

        def gemm_pair(layer, p):
            """Input-projection GEMM for steps 2p, 2p+1 -> new psum generation."""
            if layer == 0:
                ps = ps0p.tile([H, 512], f32, tag="ps0")
                wt, rhs = wih0, xaug[:, p * 2 * BL : (p + 1) * 2 * BL]
            else:
                ps = ps1p.tile([H, 512], f32, tag="ps1")
                s = (2 * p) % R0
                wt, rhs = wih1, h0ring[:, s * BL : s * BL + 2 * BL]
            pair_ps[layer][p] = ps
            if p >= 2:
                pair_ps[layer].pop(p - 2, None)
            for g in range(3):
                nc.tensor.matmul(
                    ps[:, g * 128 : (g + 1) * 128],
                    wt[:, g * 128 : (g + 1) * 128],
                    rhs,
                    start=(g == 0),
                    stop=False,
                    skip_group_check=True,
                )
            return ps

        def step(layer, t):
            toff = t % 2
            ps = pair_ps[layer][t // 2]
            whh = whh0 if layer == 0 else whh1
            if layer == 0:
                hprev = (
                    h0ring[:, ((t - 1) % R0) * BL : ((t - 1) % R0) * BL + BL]
                    if t > 0
                    else None
                )
            else:
                hprev = (
                    h1ring[:, ((t - 1) % R1) * BL : ((t - 1) % R1) * BL + BL]
                    if t > 0
                    else None
                )
            if hprev is not None:
                for g, base in ((0, 0), (1, 128), (2, 384)):
                    nc.tensor.matmul(
                        ps[:, base + toff * 64 : base + toff * 64 + 64],
                        whh[:, g * 128 : (g + 1) * 128],
                        hprev,
                        start=False,
                        stop=False,
                        skip_group_check=True,
                    )

            sfx = f"{layer}"
            rz = wk.tile([H, 128], bf16, tag="rz" + sfx)
            if layer == 0:
                sig_in = ps[:].rearrange("p (a b c) -> p a b c", a=4, b=2, c=64)[
                    :, 0:2, toff, :
                ]
                sig_out = rz[:].rearrange("p (g c) -> p g c", g=2, c=64)
                nc.scalar.activation(sig_out, sig_in, Act.Sigmoid)
            else:
                nc.scalar.activation(
                    rz[:, 0:64],
                    ps[:, toff * 64 : toff * 64 + 64],
                    Act.Sigmoid,
                    bias=br1,
                )
                nc.scalar.activation(
                    rz[:, 64:128],
                    ps[:, 128 + toff * 64 : 128 + toff * 64 + 64],
                    Act.Sigmoid,
                    bias=bz1,
                )
            r = rz[:, 0:64]
            z = rz[:, 64:128]
            bhn = bhn0 if layer == 0 else bhn1

            u = wk.tile([H, BL], bf16, tag="u" + sfx)
            if t > 0:
                nc.vector.scalar_tensor_tensor(
                    u[:],
                    ps[:, 384 + toff * 64 : 384 + toff * 64 + 64],
                    bhn,
                    r,
                    op0=Alu.add,
                    op1=Alu.mult,
                )
            else:
                nc.vector.tensor_scalar_mul(u[:], r, bhn)
            u2 = wk.tile([H, BL], bf16, tag="u2" + sfx)
            nc.vector.tensor_tensor(
                u2[:], u[:], ps[:, 256 + toff * 64 : 256 + toff * 64 + 64], op=Alu.add
            )
            n = wk.tile([H, BL], bf16, tag="n" + sfx)
            if layer == 0:
                nc.scalar.activation(n[:], u2[:], Act.Tanh)
            else:
                nc.scalar.activation(n[:], u2[:], Act.Tanh, bias=bin1)

            v = wk.tile([H, BL], bf16, tag="v" + sfx)
            if hprev is not None:
                nc.gpsimd.tensor_sub(v[:], hprev, n[:])
            else:
                nc.gpsimd.tensor_scalar_mul(v[:], n[:], -1.0)
            w = wk.tile([H, BL], bf16, tag="w" + sfx)
            nc.gpsimd.tensor_mul(w[:], z, v[:])

            if layer == 0:
                hnew = h0ring[:, (t % R0) * BL : (t % R0) * BL + BL]
            else:
                hnew = h1ring[:, (t % R1) * BL : (t % R1) * BL + BL]
            nc.vector.tensor_tensor(hnew, n[:], w[:], op=Alu.add)

        # ---- main recurrence, layer 1 lags by 2 steps ----
        for t in range(t_steps):
            if t % 2 == 0:
                gemm_pair(0, t // 2)
            step(0, t)
            if t % 2 == 1:
                gemm_pair(1, (t - 1) // 2)
            if t >= 2:
                step(1, t - 2)
        step(1, t_steps - 2)
        step(1, t_steps - 1)

        # ---- MLP head ----
        # hid[k, f*64+b] = relu(W1h @ h1 + W1f @ xfut + b1)
        hlast = h1ring[:, ((t_steps - 1) % R1) * BL : ((t_steps - 1) % R1) * BL + BL]
        hdbg = wk.tile([H, BL], f32, tag="hdbg")
        nc.scalar.copy(hdbg[:], hlast)
        nc.sync.dma_start(d_hdbg, hdbg[:])
        h0last = h0ring[:, ((t_steps - 1) % R0) * BL : ((t_steps - 1) % R0) * BL + BL]
        hdbg0 = wk.tile([H, BL], f32, tag="hdbg0")
        nc.scalar.copy(hdbg0[:], h0last)
        nc.sync.dma_start(d_hdbg0, hdbg0[:])
        ysb = wk.tile([1, BL * FH], f32, tag="ysb")
        for blk in range(3):
            psm = ps0p.tile([H, 512], f32, tag="ps0")
            for f in range(8):
                nc.tensor.matmul(
                    psm[:, f * 64 : (f + 1) * 64],
                    w1h[:],
                    hlast,
                    start=(f == 0),
                    stop=False,
                    skip_group_check=True,
                )
            nc.tensor.matmul(
                psm[:],
                w1f[:],
                xfut[:, blk * 512 : (blk + 1) * 512],
                start=False,
                stop=True,
                skip_group_check=True,
            )
            hid = wk.tile([H, 512], bf16, tag="hid")
            nc.scalar.activation(hid[:], psm[:], Act.Relu, bias=b1v)
            yps = ps1p.tile([1, 512], f32, tag="yps")
            nc.tensor.matmul(
                yps[:], w2[:], hid[:], start=True, stop=True, skip_group_check=True
            )
            nc.scalar.activation(
                ysb[:, blk * 512 : (blk + 1) * 512],
                yps[:],
                Act.Identity,
                bias=b2,
            )
        nc.sync.dma_start(d_y, ysb[:])

        for p in (ps1p, ps0p, wk, ring, const):
            p.release()

    nc.compile()
    return nc


def _host_prep(inputs, t_steps, n_cores):
    import ml_dtypes

    bf16 = ml_dtypes.bfloat16

    def f32(x):
        return np.asarray(x, dtype=np.float32)

    x_enc = f32(inputs["x_enc"])[:, :t_steps, :]
    x_fut = f32(inputs["x_future_time"])
    W_ih0, W_hh0 = f32(inputs["W_ih0"]), f32(inputs["W_hh0"])
    b_ih0, b_hh0 = f32(inputs["b_ih0"]), f32(inputs["b_hh0"])
    W_ih1, W_hh1 = f32(inputs["W_ih1"]), f32(inputs["W_hh1"])
    b_ih1, b_hh1 = f32(inputs["b_ih1"]), f32(inputs["b_hh1"])
    W1, b1 = f32(inputs["W1"]), f32(inputs["b1"])
    W2, b2 = f32(inputs["W2"]), f32(inputs["b2"])

    bl = x_enc.shape[0] // n_cores

    # wih0 augmented with a bias row (b_ih + b_hh for r,z; b_ih only for n)
    brow = b_ih0 + b_hh0
    brow[2 * H :] = b_ih0[2 * H :]
    wih0aug = np.concatenate([W_ih0.T, brow[None, :]], axis=0).astype(bf16)  # [17,3H]

    bias = np.zeros((H, 9), dtype=np.float32)
    bias[:, 0] = b_hh0[2 * H :]
    bias[:, 1] = (b_ih1 + b_hh1)[:H]
    bias[:, 2] = (b_ih1 + b_hh1)[H : 2 * H]
    bias[:, 3] = b_hh1[2 * H :]
    bias[:, 4] = b_ih1[2 * H :]
    bias[:, 5] = b1
    bias[:, 6] = b2[0]

    wpack = np.zeros((128, 4 * 384 + 128 + 128 + 1), dtype=bf16)
    wpack[:, 0:384] = W_hh0.T.astype(bf16)
    wpack[:, 384:768] = W_ih1.T.astype(bf16)
    wpack[:, 768:1152] = W_hh1.T.astype(bf16)
    wpack[: E + 1, 1152:1536] = wih0aug
    wpack[:, 1536:1664] = W1[:, :H].T.astype(bf16)
    wpack[:FT, 1664:1792] = W1[:, H:].T.astype(bf16)
    wpack[:, 1792:1793] = W2.reshape(128, 1).astype(bf16)

    shared = {
        "wpack": wpack,
        "bias": bias,
    }

    in_maps = []
    for c in range(n_cores):
        xl = x_enc[c * bl : (c + 1) * bl]  # [bl, T, E]
        # cols = t*bl + b ; rows = feature (+ ones row)
        xT = xl.transpose(2, 1, 0).reshape(E, t_steps * bl)
        xaug = np.concatenate(
            [xT, np.ones((1, t_steps * bl), dtype=np.float32)], axis=0
        ).astype(bf16)
        xf = x_fut[c * bl : (c + 1) * bl]  # [bl, FH, FT]
        # cols = f*bl + b
        xfT = xf.transpose(2, 1, 0).reshape(FT, FH * bl).astype(bf16)
        m = dict(shared)
        m["xaug"] = xaug
        m["xfut"] = xfT
        in_maps.append(m)
    return in_maps


def _run(inputs, t_steps, trace=False):
    from concourse.bass_utils import run_bass_kernel_spmd

    key = t_steps
    if key not in _PROG_CACHE:
        _PROG_CACHE[key] = _build_program(t_steps)
    nc = _PROG_CACHE[key]
    in_maps = _host_prep(inputs, t_steps, NCORES)
    res = run_bass_kernel_spmd(nc, in_maps, list(range(NCORES)), trace=trace)
    bl = BL
    y = np.zeros((NCORES * bl, FH), dtype=np.float32)
    for c in range(NCORES):
        yc = np.asarray(res.results[c]["y"], dtype=np.float32).reshape(FH, bl)
        y[c * bl : (c + 1) * bl] = yc.T
    return y, res


def kernel(**inputs):
    y, _ = _run(inputs, T)
    return y


# revision 44
# speedup vs baseline: 3235.5543x; 3235.5543x over previous
import numpy as np

# nn_GRUDirectModel on 8 trn2 cores.
# 2-layer GRU (PyTorch gate order r,z,n) + MLP head.
# B=512, T=336, E=16, H=128, FH=24, FT=4.
# Data-parallel: batch sharded 8 x 64. Per core, the recurrence runs in a
# "gate-partition" layout: hidden state h^T is [H=128 partitions, 64 batch],
# gate pre-activations live in PSUM, where chunked input-projection GEMMs
# pre-fill xp and the per-step hidden matmuls accumulate onto them.

B, T, E, H, FH, FT = 512, 336, 16, 128, 24, 4
NCORES = 8
BL = B // NCORES  # 64

_PROG_CACHE = {}


def _build_program(t_steps, reps=1):
    from concourse import bass, bacc, tile

    mybir = bass.mybir
    dt = mybir.dt
    f32 = dt.float32
    bf16 = dt.bfloat16
    Alu = mybir.AluOpType
    Act = mybir.ActivationFunctionType

    npairs = t_steps // 2
    xcols = t_steps * BL

    nc = bacc.Bacc("TRN2", target_bir_lowering=False, debug=False)

    # DRAM I/O (per-core shapes). All weights packed into one tensor so a
    # matmul never waits on more than ~2 DMA queue semaphores (ISA limit).
    WPACK_COLS = 4 * 384 + 128 + 128 + 1 + 128 + 128 + 512 + 128
    d_xaug = nc.dram_tensor("xaug", [E + 1, xcols], bf16, kind="ExternalInput").ap()
    d_xfut = nc.dram_tensor("xfut", [FT, BL * FH], bf16, kind="ExternalInput").ap()
    d_wpack = nc.dram_tensor("wpack", [128, WPACK_COLS], bf16, kind="ExternalInput").ap()
    d_bias = nc.dram_tensor("bias", [H, 9], f32, kind="ExternalInput").ap()
    d_y = nc.dram_tensor("y", [1, BL * FH], f32, kind="ExternalOutput").ap()
    d_hdbg = nc.dram_tensor("hdbg", [H, BL], f32, kind="ExternalOutput").ap()
    d_hdbg0 = nc.dram_tensor("hdbg0", [H, BL], f32, kind="ExternalOutput").ap()

    with tile.TileContext(nc) as tc:
        const = tc.alloc_tile_pool(name="const", bufs=1)
        ring = tc.alloc_tile_pool(name="ring", bufs=1)
        wk = tc.alloc_tile_pool(name="wk", bufs=8)
        ps0p = tc.alloc_tile_pool(name="ps0p", bufs=3, space="PSUM")
        ps1p = tc.alloc_tile_pool(name="ps1p", bufs=3, space="PSUM")

        xaug = const.tile([E + 1, xcols], bf16)
        xfut = const.tile([FT, BL * FH], bf16)
        wpack = const.tile([128, WPACK_COLS], bf16)
        bias = const.tile([H, 9], f32)

        # gpsimd DMAs go through the single SW-DGE queue -> one semaphore, so
        # consuming matmuls never exceed the per-instruction sync-wait limit.
        nc.gpsimd.dma_start(wpack[:], d_wpack)
        nc.gpsimd.dma_start(bias[:], d_bias)
        # x in chunks so the first GEMM isn't gated on the whole transfer
        nxc = 8 if (xcols % 8 == 0 and (xcols // 8) % 128 == 0 and xcols // 8 >= 2048) else 1
        ccols = xcols // nxc
        for j in range(nxc):
            nc.gpsimd.dma_start(
                xaug[:, j * ccols : (j + 1) * ccols],
                d_xaug[:, j * ccols : (j + 1) * ccols],
            )
        nc.gpsimd.dma_start(xfut[:], d_xfut)

        whh0 = wpack[:, 0:384]
        wih1 = wpack[:, 384:768]
        whh1 = wpack[:, 768:1152]
        wih0 = wpack[0 : E + 1, 1152:1536]
        w1h = wpack[:, 1536:1664]
        w1f = wpack[0:FT, 1664:1792]
        w2 = wpack[:, 1792:1793]
        bhn0row = wpack[0:1, 1793:1921]
        bias4 = wpack[0:4, 1921:2049]
        sel4 = wpack[0:4, 2049:2561]
        ones128 = wpack[0:1, 2561:2689]

        bhn0 = bias[:, 0:1]
        br1 = bias[:, 1:2]
        bz1 = bias[:, 2:3]
        bhn1 = bias[:, 3:4]
        bin1 = bias[:, 4:5]
        b1v = bias[:, 5:6]
        b2 = bias[0:1, 6:7]

        R0 = 8  # layer-0 output ring (slots of BL cols)
        R1 = 4
        h0ring = ring.tile([H, R0 * BL], bf16)
        h1ring = ring.tile([H, R1 * BL], bf16)

        pair_ps = [{}, {}]  # psum generation per (layer, pair)
        pair_xn = [{}, {}]

        # PSUM bank col layout (fp32, 512 cols):
        #   [0:128]  r  (2 steps x 64)
        #   [128:256] z
        #   [256:384] xn   (input-proj only; never touched by hidden MM)
        #   [384:512] hn   (hidden MM only)

        def gemm_pair(layer, p):
            """Input-projection GEMM for steps 2p, 2p+1 -> new psum generation."""
            if layer == 0:
                ps = ps0p.tile([H, 512], f32, tag="ps0")
                wt, rhs = wih0, xaug[:, p * 2 * BL : (p + 1) * 2 * BL]
            else:
                ps = ps1p.tile([H, 512], f32, tag="ps1")
                s = (2 * p) % R0
                wt, rhs = wih1, h0ring[:, s * BL : s * BL + 2 * BL]
            pair_ps[layer][p] = ps
            if p >= 2:
                pair_ps[layer].pop(p - 2, None)
            for g in range(3):
                nc.tensor.matmul(
                    ps[:, g * 128 : (g + 1) * 128],
                    wt[:, g * 128 : (g + 1) * 128],
                    rhs,
                    start=(g == 0),
                    stop=False,
                    skip_group_check=True,
                )
            if layer == 0:
                # b_hh0_n broadcast into the hn slice (K=1 matmul)
                nc.tensor.matmul(
                    ps[:, 384:512], bhn0row, ones128,
                    start=False, stop=False, skip_group_check=True,
                )
            else:
                # all four layer-1 gate biases via a K=4 selector matmul
                nc.tensor.matmul(
                    ps[:], bias4, sel4,
                    start=False, stop=False, skip_group_check=True,
                )
            # evacuate xn to SBUF (off the critical chain) so the u2 add is a
            # cheap bf16 SBUF-only op instead of a PSUM-source one
            xn = wk.tile([H, 128], bf16, tag="xn" + str(layer))
            nc.scalar.activation(xn[:], ps[:, 256:384], Act.Copy)
            pair_xn[layer][p] = xn
            return ps

        def step_gen(layer, t):
            # Generator yielding between engine phases so the driver can
            # interleave the two layers' chains (software pipelining).
            toff = t % 2
            ps = pair_ps[layer][t // 2]
            whh = whh0 if layer == 0 else whh1
            if layer == 0:
                hprev = (
                    h0ring[:, ((t - 1) % R0) * BL : ((t - 1) % R0) * BL + BL]
                    if t > 0
                    else None
                )
            else:
                hprev = (
                    h1ring[:, ((t - 1) % R1) * BL : ((t - 1) % R1) * BL + BL]
                    if t > 0
                    else None
                )
            if hprev is not None:
                for g, base in ((2, 384), (0, 0), (1, 128)):
                    nc.tensor.matmul(
                        ps[:, base + toff * 64 : base + toff * 64 + 64],
                        whh[:, g * 128 : (g + 1) * 128],
                        hprev,
                        start=False,
                        stop=False,
                        skip_group_check=True,
                    )
            yield

            sfx = f"{layer}"
            rz = wk.tile([H, 128], bf16, tag="rz" + sfx)
            sig_in = ps[:].rearrange("p (a b c) -> p a b c", a=4, b=2, c=64)[
                :, 0:2, toff, :
            ]
            sig_out = rz[:].rearrange("p (g c) -> p g c", g=2, c=64)
            nc.scalar.activation(sig_out, sig_in, Act.Sigmoid)
            r = rz[:, 0:64]
            z = rz[:, 64:128]
            yield

            # n-path: u = hn_psum (incl. b_hn) * r ; u2 = u + xn_psum
            # back-to-back on DVE: no cross-engine hop between them
            u = wk.tile([H, BL], bf16, tag="u" + sfx)
            nc.vector.tensor_tensor(
                u[:], ps[:, 384 + toff * 64 : 384 + toff * 64 + 64], r, op=Alu.mult
            )
            u2 = wk.tile([H, BL], bf16, tag="u2" + sfx)
            xn = pair_xn[layer][t // 2]
            nc.vector.tensor_tensor(
                u2[:], u[:], xn[:, toff * 64 : toff * 64 + 64], op=Alu.add
            )
            yield
            # off-chain on Pool: zh = z*h_prev and zc = 1-z
            zh = None
            if hprev is not None:
                zh = wk.tile([H, BL], bf16, tag="zh" + sfx)
                nc.gpsimd.tensor_mul(zh[:], z, hprev)
            zc = wk.tile([H, BL], bf16, tag="zc" + sfx)
            nc.gpsimd.tensor_scalar(
                zc[:], z, -1.0, 1.0, op0=Alu.mult, op1=Alu.add
            )
            yield
            n = wk.tile([H, BL], bf16, tag="n" + sfx)
            nc.scalar.activation(n[:], u2[:], Act.Tanh)
            yield
            # h_new = n*(1-z) + z*h_prev ; two DVE ops back-to-back
            if layer == 0:
                hnew = h0ring[:, (t % R0) * BL : (t % R0) * BL + BL]
            else:
                hnew = h1ring[:, (t % R1) * BL : (t % R1) * BL + BL]
            if zh is not None:
                b_ = wk.tile([H, BL], bf16, tag="b" + sfx)
                nc.vector.tensor_tensor(b_[:], n[:], zc[:], op=Alu.mult)
                nc.vector.tensor_tensor(hnew, b_[:], zh[:], op=Alu.add)
            else:
                nc.vector.tensor_tensor(hnew, n[:], zc[:], op=Alu.mult)

        def run_interleaved(gens):
            alive = list(gens)
            while alive:
                for g in list(alive):
                    try:
                        next(g)
                    except StopIteration:
                        alive.remove(g)

        # ---- main recurrence, layer 1 lags by LAG steps ----
        LAG = 4
        for _rep in range(reps):
            pair_ps[0].clear()
            pair_ps[1].clear()
            for t in range(t_steps):
                if t % 2 == 0:
                    gemm_pair(0, t // 2)
                if t % 2 == 0 and 2 <= t:
                    gemm_pair(1, (t - 2) // 2)
                gens = [step_gen(0, t)]
                if t >= LAG:
                    gens.append(step_gen(1, t - LAG))
                run_interleaved(gens)
            gemm_pair(1, t_steps // 2 - 1)
            for t in range(t_steps - LAG, t_steps):
                run_interleaved([step_gen(1, t)])
            # ---- MLP head ----
            # hid[k, f*64+b] = relu(W1h @ h1 + W1f @ xfut + b1)
            hlast = h1ring[:, ((t_steps - 1) % R1) * BL : ((t_steps - 1) % R1) * BL + BL]
            hdbg = wk.tile([H, BL], f32, tag="hdbg")
            nc.scalar.copy(hdbg[:], hlast)
            nc.sync.dma_start(d_hdbg, hdbg[:])
            h0last = h0ring[:, ((t_steps - 1) % R0) * BL : ((t_steps - 1) % R0) * BL + BL]
            hdbg0 = wk.tile([H, BL], f32, tag="hdbg0")
            nc.scalar.copy(hdbg0[:], h0last)
            nc.sync.dma_start(d_hdbg0, hdbg0[:])
            ysb = wk.tile([1, BL * FH], f32, tag="ysb")
            for blk in range(3):
                psm = ps0p.tile([H, 512], f32, tag="ps0")
                for f in range(8):
                    nc.tensor.matmul(
                        psm[:, f * 64 : (f + 1) * 64],
                        w1h[:],
                        hlast,
                        start=(f == 0),
                        stop=False,
                        skip_group_check=True,
                    )
                nc.tensor.matmul(
                    psm[:],
                    w1f[:],
                    xfut[:, blk * 512 : (blk + 1) * 512],
                    start=False,
                    stop=True,
                    skip_group_check=True,
                )
                hid = wk.tile([H, 512], bf16, tag="hid")
                nc.scalar.activation(hid[:], psm[:], Act.Relu, bias=b1v)
                yps = ps1p.tile([1, 512], f32, tag="ps1")
                nc.tensor.matmul(
                    yps[:], w2[:], hid[:], start=True, stop=True, skip_group_check=True
                )
                nc.scalar.activation(
                    ysb[:, blk * 512 : (blk + 1) * 512],
                    yps[:],
                    Act.Identity,
                    bias=b2,
                )
            nc.sync.dma_start(d_y, ysb[:])



        for p in (ps1p, ps0p, wk, ring, const):
            p.release()

    nc.compile()
    return nc


def _host_prep(inputs, t_steps, n_cores):
    import ml_dtypes

    bf16 = ml_dtypes.bfloat16

    def f32(x):
        return np.asarray(x, dtype=np.float32)

    x_enc = f32(inputs["x_enc"])[:, :t_steps, :]
    x_fut = f32(inputs["x_future_time"])
    W_ih0, W_hh0 = f32(inputs["W_ih0"]), f32(inputs["W_hh0"])
    b_ih0, b_hh0 = f32(inputs["b_ih0"]), f32(inputs["b_hh0"])
    W_ih1, W_hh1 = f32(inputs["W_ih1"]), f32(inputs["W_hh1"])
    b_ih1, b_hh1 = f32(inputs["b_ih1"]), f32(inputs["b_hh1"])
    W1, b1 = f32(inputs["W1"]), f32(inputs["b1"])
    W2, b2 = f32(inputs["W2"]), f32(inputs["b2"])

    bl = x_enc.shape[0] // n_cores

    # wih0 augmented with a bias row (b_ih + b_hh for r,z; b_ih only for n)
    brow = b_ih0 + b_hh0
    brow[2 * H :] = b_ih0[2 * H :]
    wih0aug = np.concatenate([W_ih0.T, brow[None, :]], axis=0).astype(bf16)  # [17,3H]

    bias = np.zeros((H, 9), dtype=np.float32)
    bias[:, 0] = b_hh0[2 * H :]
    bias[:, 1] = (b_ih1 + b_hh1)[:H]
    bias[:, 2] = (b_ih1 + b_hh1)[H : 2 * H]
    bias[:, 3] = b_hh1[2 * H :]
    bias[:, 4] = b_ih1[2 * H :]
    bias[:, 5] = b1
    bias[:, 6] = b2[0]

    wpack = np.zeros((128, 4 * 384 + 128 + 128 + 1 + 128 + 128 + 512 + 128), dtype=bf16)
    wpack[:, 0:384] = W_hh0.T.astype(bf16)
    wpack[:, 384:768] = W_ih1.T.astype(bf16)
    wpack[:, 768:1152] = W_hh1.T.astype(bf16)
    wpack[: E + 1, 1152:1536] = wih0aug
    wpack[:, 1536:1664] = W1[:, :H].T.astype(bf16)
    wpack[:FT, 1664:1792] = W1[:, H:].T.astype(bf16)
    wpack[:, 1792:1793] = W2.reshape(128, 1).astype(bf16)
    wpack[0, 1793:1921] = b_hh0[2 * H :].astype(bf16)
    wpack[0, 1921:2049] = (b_ih1 + b_hh1)[:H].astype(bf16)
    wpack[1, 1921:2049] = (b_ih1 + b_hh1)[H : 2 * H].astype(bf16)
    wpack[2, 1921:2049] = b_ih1[2 * H :].astype(bf16)
    wpack[3, 1921:2049] = b_hh1[2 * H :].astype(bf16)
    for k in range(4):
        wpack[k, 2049 + k * 128 : 2049 + (k + 1) * 128] = 1.0
    wpack[0, 2561:2689] = 1.0

    shared = {
        "wpack": wpack,
        "bias": bias,
    }

    in_maps = []
    for c in range(n_cores):
        xl = x_enc[c * bl : (c + 1) * bl]  # [bl, T, E]
        # cols = t*bl + b ; rows = feature (+ ones row)
        xT = xl.transpose(2, 1, 0).reshape(E, t_steps * bl)
        xaug = np.concatenate(
            [xT, np.ones((1, t_steps * bl), dtype=np.float32)], axis=0
        ).astype(bf16)
        xf = x_fut[c * bl : (c + 1) * bl]  # [bl, FH, FT]
        # cols = f*bl + b
        xfT = xf.transpose(2, 1, 0).reshape(FT, FH * bl).astype(bf16)
        m = dict(shared)
        m["xaug"] = xaug
        m["xfut"] = xfT
        in_maps.append(m)
    return in_maps


def _run(inputs, t_steps, trace=False):
    from concourse.bass_utils import run_bass_kernel_spmd

    key = t_steps
    if key not in _PROG_CACHE:
        _PROG_CACHE[key] = _build_program(t_steps)
    nc = _PROG_CACHE[key]
    in_maps = _host_prep(inputs, t_steps, NCORES)
    res = run_bass_kernel_spmd(nc, in_maps, list(range(NCORES)), trace=trace)
    bl = BL
    y = np.zeros((NCORES * bl, FH), dtype=np.float32)
    for c in range(NCORES):
        yc = np.asarray(res.results[c]["y"], dtype=np.float32).reshape(FH, bl)
        y[c * bl : (c + 1) * bl] = yc.T
    return y, res


def kernel(**inputs):
    y, _ = _run(inputs, T)
    return y

